# revision 1
# baseline (speedup 1.0000x reference)
"""Trainium2 Bass kernel for nn_AttentionResidualBlock.

Computation (per token t, head h):
    q = x @ W_q + b_q
    scores[t,h,l] = <q[t,h,:], k[t,l,h,:]> / sqrt(hd)   (k = layer_history)
    w = softmax_l(scores)
    out[t,h,:] = sum_l w[t,h,l] * k[t,l,h,:]

Sharding: data-parallel over the 8192 (b,s) tokens -> 8 cores x 1024 tokens.
Per-core layout: token-major (tokens on partitions), 8 tiles of 128 tokens.

Per tile:
  - layer_history arrives as bf16 via SWDGE cast-DMA (same HBM traffic,
    half the SBUF, and enables the DVE 2x_1P tensor_tensor mode)
  - q_proj on PE in fp32r (full rate at n=512, ~fp32 precision): the x tile
    is transposed with PE transposes, then 16 accumulating matmuls plus a
    k=1 "ones" matmul adds b_q; ACT copies PSUM->SBUF as bf16 with the
    1/sqrt(hd) scale folded in
  - scores: one DVE bf16 mul (q broadcast over l via a step-0 AP dim), then
    an in-place pairwise fold tree over hd (bf16 2x) with an fp32 tail
  - softmax over l=12 without max subtraction (scores ~ N(0,1))
  - normalized weights are written as bf16 pairs and broadcast across hd
    with step-0-source fp32-word copies on ACT, pipelined in 6 groups of 2
    layers with the weighted-sum muls so the serial ACT prefix stays short
  - weighted sum: DVE bf16 mul into a double-buffered product tile; the sum
    over l runs on PE as accumulating identity-matmul copies into PSUM
    (exact fp32 accumulation), with the PSUM->SBUF output drain deferred by
    one tile so ACT never stalls on the PE engine-counter semaphore
DVE is the bottleneck engine (~22 us/tile busy); DMA ~21 us/tile, PE ~17,
ACT ~11. Measured ~193 us/core on HW median (DMA roofline ~165 us).
"""

import math
from contextlib import ExitStack

import numpy as np

import concourse.tile as tile
from concourse import bacc, mybir
from concourse.bass_utils import run_bass_kernel_spmd
from concourse import masks

FP32 = mybir.dt.float32
FP32R = mybir.dt.float32r
BF16 = mybir.dt.bfloat16

B, S, L, D, H = 4, 2048, 12, 1024, 16
HD = D // H
N_CORES = 8
T = B * S // N_CORES          # tokens per core = 1024
P = 128                       # partition tile
NT = T // P                   # 8 token tiles per core
SCALE = 1.0 / math.sqrt(HD)   # 0.125


def build_body(ctx, tc, out, xt, kh, wq, bq, ones, repeat=1):
    nc = tc.nc

    const_pool = ctx.enter_context(tc.tile_pool(name="const", bufs=1))
    # W as lhsT chunks: w_sb[p, c, j] = W[c*128 + p, j]
    w_sb = const_pool.tile([P, 8, D], FP32R)
    wqr = wq.rearrange("(c p) j -> p c j", p=P).bitcast(FP32R)
    nc.scalar.dma_start(w_sb[:, :, 0:512], wqr[:, :, 0:512])
    nc.scalar.dma_start(w_sb[:, :, 512:1024], wqr[:, :, 512:1024])
    bq_sb = const_pool.tile([1, D], BF16)
    nc.gpsimd.dma_start(bq_sb[:], bq.unsqueeze(0))
    ones_sb = const_pool.tile([1, P], BF16)
    nc.gpsimd.dma_start(ones_sb[:], ones.unsqueeze(0))
    ident_bf = const_pool.tile([P, P], BF16)
    masks.make_identity(nc, ident_bf[:])

    kp = ctx.enter_context(tc.tile_pool(name="k", bufs=2))
    xtp = ctx.enter_context(tc.tile_pool(name="xt", bufs=2))
    qp = ctx.enter_context(tc.tile_pool(name="q", bufs=2))
    prodp = ctx.enter_context(tc.tile_pool(name="prod", bufs=1))
    p2p = ctx.enter_context(tc.tile_pool(name="p2", bufs=2))
    wbp = ctx.enter_context(tc.tile_pool(name="wb", bufs=1))
    sp = ctx.enter_context(tc.tile_pool(name="smx", bufs=2))
    ps_t = ctx.enter_context(tc.tile_pool(name="ps_t", bufs=2, space="PSUM"))
    ps_q = ctx.enter_context(tc.tile_pool(name="ps_q", bufs=1, space="PSUM"))
    ps_a = ctx.enter_context(tc.tile_pool(name="ps_a", bufs=2, space="PSUM"))

    # PE warm-up: ~4us of dummy matmuls at t~0 so the HAM clock-gate opens
    # before tile 0's q_proj (cold PE is the prologue critical path)
    warm_ps = ps_t.tile([P, P], FP32, tag="xtps")
    for i in range(32):
        nc.tensor.matmul(
            warm_ps[:], lhsT=ident_bf[:], rhs=ident_bf[:],
            start=(i == 0), stop=(i == 31),
        )

    def flush_pending(pending):
        # one-tile-deferred output drain: by now the PE sum-over-l matmuls
        # for that tile are long done, so ACT never stalls on the PE counter
        a_prev, tok_prev = pending
        o_sb = xtp.tile([P, D], FP32, tag="xt")
        nc.scalar.copy(o_sb[:], a_prev[:])
        nc.sync.dma_start(out[tok_prev], o_sb[:])

    pending = None
    for tt in range(NT * repeat):
        tt = tt % NT
        tok = slice(tt * P, (tt + 1) * P)

        # ---- loads ----
        k_bf = kp.tile([P, L, D], BF16, tag="k")
        if tt == 0:
            # split the first k load so tile 0's scores can start earlier
            nc.gpsimd.dma_start(k_bf[:, 0:6, :], kh[tok, 0:6, :])
            nc.gpsimd.dma_start(k_bf[:, 6:12, :], kh[tok, 6:12, :])
        else:
            nc.gpsimd.dma_start(k_bf[:], kh[tok])  # fp32 -> bf16 cast DMA
        # The whole q-production chain runs at high priority so the PE order
        # places it ahead of the previous tile's sum-over-l matmuls (the
        # scores-mul waits on q through engine-counter semaphores).
        with tc.high_priority(offset=180):
            # x arrives pre-transposed from the host: xt_sb[p, c, t]
            xt_sb = xtp.tile([P, 8, P], FP32R, tag="xt")
            nc.sync.dma_start(
                xt_sb[:],
                xt[:, tok].rearrange("(c p) t -> p c t", p=P).bitcast(FP32R),
            )

            # q = x @ W + b (token-major PSUM [t, d_out])
            q_ps = ps_q.tile([P, D], FP32, tag="qps")
            for half in range(2):
                n0 = half * 512
                for c in range(8):
                    nc.tensor.matmul(
                        q_ps[:, n0:n0 + 512],
                        lhsT=xt_sb[:, c, :],
                        rhs=w_sb[:, c, n0:n0 + 512],
                        start=(c == 0),
                        stop=False,
                    )
                nc.tensor.matmul(
                    q_ps[:, n0:n0 + 512],
                    lhsT=ones_sb[:],
                    rhs=bq_sb[:, n0:n0 + 512],
                    start=False,
                    stop=True,
                )
            # q -> SBUF bf16, folding in 1/sqrt(hd), on DVE so the scores-mul
            # only waits on DVE program order for it
            q_bf = qp.tile([P, D], BF16, tag="q")
            nc.vector.tensor_scalar_mul(q_bf[:], q_ps[:], SCALE)

        if pending is not None:
            flush_pending(pending)
            pending = None

        # ---- scores: prod = k * q (broadcast over l), fold-reduce over hd ----
        k4 = k_bf[:].rearrange("p l (h e) -> p l h e", h=H)
        qv = (
            q_bf[:]
            .rearrange("p (h e) -> p h e", h=H)
            .unsqueeze(1)
            .broadcast_to([P, L, H, HD])
        )
        prod = prodp.tile([P, L, H, HD], BF16, tag="prod")
        scr = sp.tile([P, L, H], FP32, tag="scr")
        with tc.high_priority(offset=60):
            # tile 0: two l-halves so compute starts as soon as half of k is in
            for ls in ([slice(0, 6), slice(6, 12)] if tt == 0 else [slice(0, L)]):
                nl = ls.stop - ls.start
                nc.vector.tensor_mul(prod[:, ls], k4[:, ls], qv[:, ls])
                # in-place fold tree over hd: 64->32->...->2, then fp32 tail add.
                # dst aliases in1 exactly (same element positions) which is safe
                # for the streaming DVE.
                off = 0
                for w0 in (32, 16, 8, 4, 2):
                    nc.vector.tensor_add(
                        prod[:, ls, :, off + w0:off + 2 * w0],
                        prod[:, ls, :, off:off + w0],
                        prod[:, ls, :, off + w0:off + 2 * w0],
                    )
                    off += w0
                # off == 62: two surviving partials at 62, 63
                nc.vector.tensor_add(
                    scr[:, ls].unsqueeze(3),
                    prod[:, ls, :, 62:63],
                    prod[:, ls, :, 63:64],
                )


        # ---- softmax over l (no max subtraction) ----
        es = sp.tile([P, L, H], FP32, tag="es")
        nc.scalar.activation(es[:], scr[:], mybir.ActivationFunctionType.Exp)
        den = sp.tile([P, H], FP32, tag="den")
        nc.vector.tensor_reduce(
            den[:],
            es[:].rearrange("p l h -> p h l"),
            axis=mybir.AxisListType.X,
            op=mybir.AluOpType.add,
        )
        rd = sp.tile([P, H], FP32, tag="rd")
        nc.vector.reciprocal(rd[:], den[:])

        # normalized weights into slots 0,1 of the expanded tile (a bf16
        # pair = one fp32 word), then fp32-word doubling broadcast on ACT.
        wb = wbp.tile([P, L, H, HD], BF16, tag="wb")
        rdv = rd[:].unsqueeze(1).broadcast_to([P, L, H]).unsqueeze(3)
        # two halves so the first expansion copies can start half a norm early
        for ns_ in (slice(0, 6), slice(6, 12)):
            nc.vector.tensor_mul(
                wb[:, ns_, :, 0:2],
                es[:, ns_].unsqueeze(3).broadcast_to([P, 6, H, 2]),
                rdv[:, ns_].broadcast_to([P, 6, H, 2]),
            )
        # expand each bf16 pair (one fp32 word) across hd with a single
        # step-0-source broadcast copy on ACT, split into l-halves so the
        # first half of the ws-mul overlaps the second half's expansion.
        wbf = wb[:].bitcast(FP32)  # [P, L, H, 32] fp32 words (bf16 pairs)
        prod2 = p2p.tile([P, L, D], BF16, tag="p2")
        wbflat = wb[:].rearrange("p l h e -> p l (h e)")
        acc = ps_a.tile([P, D], FP32, tag="acc")
        for lh in range(6):
            ls = slice(lh * 2, (lh + 1) * 2)
            nc.scalar.copy(
                wbf[:, ls, :, 1:32],
                wbf[:, ls, :, 0:1].broadcast_to([P, 2, H, 31]),
            )
            nc.vector.tensor_mul(
                prod2[:, ls, :], k_bf[:, ls, :], wbflat[:, ls, :]
            )
            # sum over l on PE: accumulating identity-matmul copies (fp32
            # PSUM accumulation). prod2 is double-buffered so the DVE never
            # waits on these within a tile.
            for l in range(ls.start, ls.stop):
                for half in range(2):
                    n0 = half * 512
                    nc.tensor.matmul(
                        acc[:, n0:n0 + 512],
                        lhsT=ident_bf[:],
                        rhs=prod2[:, l, n0:n0 + 512],
                        start=(l == 0),
                        stop=(l == L - 1),
                    )
        pending = (acc, tok)

    flush_pending(pending)


_NC_CACHE = {}


def build_nc(repeat=1):
    if repeat in _NC_CACHE:
        return _NC_CACHE[repeat]
    nc = bacc.Bacc("TRN2", target_bir_lowering=False, debug=False,
                   num_devices=N_CORES)
    xt = nc.dram_tensor("xt", [D, T], FP32, kind="ExternalInput").ap()
    kh = nc.dram_tensor("kh", [T, L, D], FP32, kind="ExternalInput").ap()
    wq = nc.dram_tensor("wq", [D, D], FP32, kind="ExternalInput").ap()
    bq = nc.dram_tensor("bq", [D], FP32, kind="ExternalInput").ap()
    ones = nc.dram_tensor("ones", [P], FP32, kind="ExternalInput").ap()
    out = nc.dram_tensor("out", [T, D], FP32, kind="ExternalOutput").ap()
    with tile.TileContext(nc) as tc, ExitStack() as ctx:
        build_body(ctx, tc, out, xt, kh, wq, bq, ones, repeat=repeat)
    nc.compile()
    _NC_CACHE[repeat] = nc
    return nc


def make_in_maps(x_current, layer_history, W_q, b_q):
    x_flat = np.ascontiguousarray(
        x_current.reshape(B * S, D), dtype=np.float32)
    k_flat = np.ascontiguousarray(
        layer_history.reshape(B * S, L, D), dtype=np.float32)
    W_q = np.ascontiguousarray(W_q, dtype=np.float32)
    b_q = np.ascontiguousarray(b_q, dtype=np.float32)
    in_maps = []
    for c in range(N_CORES):
        sl = slice(c * T, (c + 1) * T)
        in_maps.append({
            "xt": np.ascontiguousarray(x_flat[sl].T),
            "kh": k_flat[sl],
            "wq": W_q,
            "bq": b_q,
            "ones": np.ones((P,), np.float32),
        })
    return in_maps


def kernel(x_current, layer_history, W_q, b_q):
    nc = build_nc()
    in_maps = make_in_maps(x_current, layer_history, W_q, b_q)
    res = run_bass_kernel_spmd(nc, in_maps, core_ids=list(range(N_CORES)))
    out = np.concatenate([res.results[c]["out"] for c in range(N_CORES)], axis=0)
    return out.reshape(B, S, D).astype(np.float32)


if __name__ == "__main__":
    rng = np.random.default_rng(0)
    x = rng.standard_normal((B, S, D), dtype=np.float32)
    k = rng.standard_normal((B, S, L, D), dtype=np.float32)
    W = (rng.standard_normal((D, D), dtype=np.float32) / math.sqrt(D)).astype(np.float32)
    b = (rng.standard_normal((D,), dtype=np.float32) * 0.01).astype(np.float32)
    o = kernel(x, k, W, b)
    print("ok", o.shape, o.dtype, float(np.abs(o).mean()))



# revision 2
# speedup vs baseline: 1.2187x; 1.2187x over previous
"""Trainium2 Bass kernel for nn_AttentionResidualBlock.

Computation (per token t, head h):
    q = x @ W_q + b_q
    scores[t,h,l] = <q[t,h,:], k[t,l,h,:]> / sqrt(hd)   (k = layer_history)
    w = softmax_l(scores)
    out[t,h,:] = sum_l w[t,h,l] * k[t,l,h,:]

Sharding: data-parallel over the 8192 (b,s) tokens -> 8 cores x 1024 tokens.
Per-core layout: token-major (tokens on partitions), 8 tiles of 128 tokens.

v2 changes vs the 227us baseline:
  - all inputs are cast to fp16 on the host (layer_history was read as fp32
    and cast-DMA'd to bf16 before): HBM traffic halves (DMA ~21us/tile ->
    ~10.5us/tile) and accuracy improves (fp16 mantissa 10 bits vs bf16 7).
  - all DMA moved to HWDGE (sync engine) since no cast is needed anymore;
    GPSIMD is freed for compute and SWDGE port contention is avoided.
  - x^T is loaded once for the whole core (2MB, 2KB lines) instead of per
    tile.
  - q PSUM->SBUF scale-copy moved from DVE to ACT.
  - the hd fold-tree and the weighted-sum mul are split between DVE
    (tensor_tensor 2x_1P, never contends with the GPSIMD SBUF port) and
    GPSIMD (tensor_tensor ~2ns/elem) with tunable layer splits GF/GP2.
DVE was the bottleneck engine (~22us/tile); after the split DVE ~16us/tile
and GPSIMD ~15us/tile run concurrently.
"""

import math
from contextlib import ExitStack

import numpy as np

import concourse.tile as tile
from concourse import bacc, mybir
from concourse.bass_utils import run_bass_kernel_spmd
from concourse import masks

FP32 = mybir.dt.float32
FP16 = mybir.dt.float16

B, S, L, D, H = 4, 2048, 12, 1024, 16
HD = D // H
N_CORES = 8
T = B * S // N_CORES          # tokens per core = 1024
P = 128                       # partition tile
NT = T // P                   # 8 token tiles per core
SCALE = 1.0 / math.sqrt(HD)   # 0.125

# engine split tuning: layers handled by GPSIMD instead of DVE
GF = 4    # fold-tree layers on gpsimd (of L=12)
GP2 = 4   # weighted-sum mul layers on gpsimd (of L=12)


def build_body(ctx, tc, out, xt, kh, wq, bq, ones, repeat=1):
    nc = tc.nc

    const_pool = ctx.enter_context(tc.tile_pool(name="const", bufs=1))
    # W as lhsT chunks: w_sb[p, c, j] = W[c*128 + p, j]
    w_sb = const_pool.tile([P, 8, D], FP16)
    wqr = wq.rearrange("(c p) j -> p c j", p=P)
    nc.scalar.dma_start(w_sb[:, :, 0:512], wqr[:, :, 0:512])
    nc.scalar.dma_start(w_sb[:, :, 512:1024], wqr[:, :, 512:1024])
    bq_sb = const_pool.tile([1, D], FP16)
    nc.scalar.dma_start(bq_sb[:], bq.unsqueeze(0))
    ones_sb = const_pool.tile([1, P], FP16)
    nc.scalar.dma_start(ones_sb[:], ones.unsqueeze(0))
    ident = const_pool.tile([P, P], FP16)
    masks.make_identity(nc, ident[:])
    # x^T for the whole core, loaded once: xt_sb[p, c, t] = x[c*128+p, t]
    xt_sb = const_pool.tile([P, 8, T], FP16)
    nc.sync.dma_start(xt_sb[:, :, 0:512], xt.rearrange("(c p) t -> p c t", p=P)[:, :, 0:512])
    nc.sync.dma_start(xt_sb[:, :, 512:1024], xt.rearrange("(c p) t -> p c t", p=P)[:, :, 512:1024])

    kp = ctx.enter_context(tc.tile_pool(name="k", bufs=2))
    qp = ctx.enter_context(tc.tile_pool(name="q", bufs=2))
    prodp = ctx.enter_context(tc.tile_pool(name="prod", bufs=2))
    p2p = ctx.enter_context(tc.tile_pool(name="p2", bufs=2))
    op = ctx.enter_context(tc.tile_pool(name="o", bufs=2))
    sp = ctx.enter_context(tc.tile_pool(name="smx", bufs=2))
    ps_t = ctx.enter_context(tc.tile_pool(name="ps_t", bufs=1, space="PSUM"))
    ps_q = ctx.enter_context(tc.tile_pool(name="ps_q", bufs=1, space="PSUM"))
    ps_a = ctx.enter_context(tc.tile_pool(name="ps_a", bufs=2, space="PSUM"))

    # PE warm-up: ~4us of dummy matmuls at t~0 so the HAM clock-gate opens
    # before tile 0's q_proj (cold PE is the prologue critical path)
    warm_ps = ps_t.tile([P, P], FP32, tag="warm")
    for i in range(32):
        nc.tensor.matmul(
            warm_ps[:], lhsT=ident[:], rhs=ident[:],
            start=(i == 0), stop=(i == 31),
        )

    def flush_pending(pending):
        # one-tile-deferred output drain: by now the PE sum-over-l matmuls
        # for that tile are long done, so ACT never stalls on the PE counter
        a_prev, tok_prev = pending
        o_sb = op.tile([P, D], FP32, tag="o")
        nc.scalar.copy(o_sb[:], a_prev[:])
        nc.sync.dma_start(out[tok_prev], o_sb[:])

    pending = None
    for tt in range(NT * repeat):
        tt = tt % NT
        tok = slice(tt * P, (tt + 1) * P)

        # ---- loads ----
        k_sb = kp.tile([P, L, D], FP16, tag="k")
        if tt == 0:
            # split the first k load so tile 0's scores can start earlier
            nc.sync.dma_start(k_sb[:, 0:6, :], kh[tok, 0:6, :])
            nc.sync.dma_start(k_sb[:, 6:12, :], kh[tok, 6:12, :])
        else:
            nc.sync.dma_start(k_sb[:], kh[tok])

        # The whole q-production chain runs at high priority so the PE order
        # places it ahead of the previous tile's sum-over-l matmuls (the
        # scores-mul waits on q through engine-counter semaphores).
        with tc.high_priority(offset=180):
            # q = x @ W + b (token-major PSUM [t, d_out])
            q_ps = ps_q.tile([P, D], FP32, tag="qps")
            for half in range(2):
                n0 = half * 512
                for c in range(8):
                    nc.tensor.matmul(
                        q_ps[:, n0:n0 + 512],
                        lhsT=xt_sb[:, c, tok],
                        rhs=w_sb[:, c, n0:n0 + 512],
                        start=(c == 0),
                        stop=False,
                    )
                nc.tensor.matmul(
                    q_ps[:, n0:n0 + 512],
                    lhsT=ones_sb[:],
                    rhs=bq_sb[:, n0:n0 + 512],
                    start=False,
                    stop=True,
                )
            # q -> SBUF fp16 with the 1/sqrt(hd) scale folded in, on ACT
            # (PSUM-src copy; keeps DVE free for the tensor_tensor stream)
            q_16 = qp.tile([P, D], FP16, tag="q")
            nc.scalar.mul(q_16[:], q_ps[:], SCALE)

        if pending is not None:
            flush_pending(pending)
            pending = None

        # ---- scores: prod = k * q (broadcast over l), fold-reduce over hd ----
        k4 = k_sb[:].rearrange("p l (h e) -> p l h e", h=H)
        qv = (
            q_16[:]
            .rearrange("p (h e) -> p h e", h=H)
            .unsqueeze(1)
            .broadcast_to([P, L, H, HD])
        )
        prod = prodp.tile([P, L, H, HD], FP16, tag="prod")
        scr = sp.tile([P, L, H], FP32, tag="scr")

        def fold(eng, ls):
            # in-place fold tree over hd: 64->32->...->2, then fp32 tail add.
            # dst aliases in1 exactly (same element positions) which is safe
            # for the streaming engines.
            off = 0
            for w0 in (32, 16, 8, 4, 2):
                eng.tensor_add(
                    prod[:, ls, :, off + w0:off + 2 * w0],
                    prod[:, ls, :, off:off + w0],
                    prod[:, ls, :, off + w0:off + 2 * w0],
                )
                off += w0
            # off == 62: two surviving partials at 62, 63
            eng.tensor_add(
                scr[:, ls].unsqueeze(3),
                prod[:, ls, :, 62:63],
                prod[:, ls, :, 63:64],
            )

        # scores-mul split so the gpsimd fold of l<GF starts one op earlier
        mul_splits = [slice(0, GF), slice(GF, L)] if 0 < GF < L else [slice(0, L)]
        if tt == 0:
            mul_splits = [slice(0, 6), slice(6, 12)]
        with tc.high_priority(offset=60):
            for ls in mul_splits:
                nc.vector.tensor_mul(prod[:, ls], k4[:, ls], qv[:, ls])
            if GF > 0:
                fold(nc.gpsimd, slice(0, GF))
            if GF < L:
                fold(nc.vector, slice(GF, L))

        # ---- softmax over l (no max subtraction) ----
        es = sp.tile([P, L, H], FP32, tag="es")
        nc.scalar.activation(es[:], scr[:], mybir.ActivationFunctionType.Exp)
        den = sp.tile([P, H], FP32, tag="den")
        nc.vector.tensor_reduce(
            den[:],
            es[:].rearrange("p l h -> p h l"),
            axis=mybir.AxisListType.X,
            op=mybir.AluOpType.add,
        )
        rd = sp.tile([P, H], FP32, tag="rd")
        nc.vector.reciprocal(rd[:], den[:])

        # normalized weights into slots 0,1 of the expanded tile (an fp16
        # pair = one fp32 word), then fp32-word doubling broadcast on ACT.
        # wb aliases prod: the fold has consumed prod by now.
        wb = prod
        rdv = rd[:].unsqueeze(1).broadcast_to([P, L, H]).unsqueeze(3)
        # two halves so the first expansion copies can start half a norm early
        for ns_ in (slice(0, 6), slice(6, 12)):
            nc.vector.tensor_mul(
                wb[:, ns_, :, 0:2],
                es[:, ns_].unsqueeze(3).broadcast_to([P, 6, H, 2]),
                rdv[:, ns_].broadcast_to([P, 6, H, 2]),
            )
        # expand each fp16 pair (one fp32 word) across hd with a single
        # step-0-source broadcast copy on ACT, split into l-halves so the
        # first half of the ws-mul overlaps the second half's expansion.
        wbf = wb[:].bitcast(FP32)  # [P, L, H, 32] fp32 words (fp16 pairs)
        prod2 = p2p.tile([P, L, D], FP16, tag="p2")
        wbflat = wb[:].rearrange("p l h e -> p l (h e)")
        acc = ps_a.tile([P, D], FP32, tag="acc")

        def wsum_mm(l, first, last):
            for half in range(2):
                n0 = half * 512
                nc.tensor.matmul(
                    acc[:, n0:n0 + 512],
                    lhsT=ident[:],
                    rhs=prod2[:, l, n0:n0 + 512],
                    start=first,
                    stop=last,
                )

        # gpsimd takes layers [0, GP2) in one op (it reads the first
        # expansion half); DVE takes the rest in 2-layer groups pipelined
        # with the expansion halves.
        for lh in range(6):
            ls = slice(lh * 2, (lh + 1) * 2)
            nc.scalar.copy(
                wbf[:, ls, :, 1:32],
                wbf[:, ls, :, 0:1].broadcast_to([P, 2, H, 31]),
            )
            if lh == (GP2 + 1) // 2 - 1 and GP2 > 0:
                gs = slice(0, GP2)
                nc.gpsimd.tensor_mul(
                    prod2[:, gs, :], k_sb[:, gs, :], wbflat[:, gs, :]
                )
            dvs = slice(max(ls.start, GP2), ls.stop)
            if dvs.start < dvs.stop:
                nc.vector.tensor_mul(
                    prod2[:, dvs, :], k_sb[:, dvs, :], wbflat[:, dvs, :]
                )
            # sum over l on PE: accumulating identity-matmul copies (fp32
            # PSUM accumulation), DVE-fed layers as they appear.
            for l in range(max(ls.start, GP2), ls.stop):
                wsum_mm(l, first=(l == GP2), last=(l == L - 1 and GP2 == 0))
        # gpsimd-fed layers accumulate last (they finish latest)
        for l in range(GP2):
            wsum_mm(l, first=(GP2 == L), last=(l == GP2 - 1))
        pending = (acc, tok)

    flush_pending(pending)


_NC_CACHE = {}


def build_nc(repeat=1):
    if repeat in _NC_CACHE:
        return _NC_CACHE[repeat]
    nc = bacc.Bacc("TRN2", target_bir_lowering=False, debug=False,
                   num_devices=N_CORES)
    xt = nc.dram_tensor("xt", [D, T], FP16, kind="ExternalInput").ap()
    kh = nc.dram_tensor("kh", [T, L, D], FP16, kind="ExternalInput").ap()
    wq = nc.dram_tensor("wq", [D, D], FP16, kind="ExternalInput").ap()
    bq = nc.dram_tensor("bq", [D], FP16, kind="ExternalInput").ap()
    ones = nc.dram_tensor("ones", [P], FP16, kind="ExternalInput").ap()
    out = nc.dram_tensor("out", [T, D], FP32, kind="ExternalOutput").ap()
    with tile.TileContext(nc) as tc, ExitStack() as ctx:
        build_body(ctx, tc, out, xt, kh, wq, bq, ones, repeat=repeat)
    nc.compile()
    _NC_CACHE[repeat] = nc
    return nc


def make_in_maps(x_current, layer_history, W_q, b_q):
    x16 = np.asarray(x_current, dtype=np.float16).reshape(B * S, D)
    k16 = np.asarray(layer_history, dtype=np.float16).reshape(B * S, L, D)
    W16 = np.asarray(W_q, dtype=np.float16)
    b16 = np.asarray(b_q, dtype=np.float16)
    in_maps = []
    for c in range(N_CORES):
        sl = slice(c * T, (c + 1) * T)
        in_maps.append({
            "xt": np.ascontiguousarray(x16[sl].T),
            "kh": np.ascontiguousarray(k16[sl]),
            "wq": W16,
            "bq": b16,
            "ones": np.ones((P,), np.float16),
        })
    return in_maps


def kernel(x_current, layer_history, W_q, b_q):
    nc = build_nc()
    in_maps = make_in_maps(x_current, layer_history, W_q, b_q)
    res = run_bass_kernel_spmd(nc, in_maps, core_ids=list(range(N_CORES)))
    out = np.concatenate([res.results[c]["out"] for c in range(N_CORES)], axis=0)
    return out.reshape(B, S, D).astype(np.float32)


if __name__ == "__main__":
    rng = np.random.default_rng(0)
    x = rng.standard_normal((B, S, D), dtype=np.float32)
    k = rng.standard_normal((B, S, L, D), dtype=np.float32)
    W = (rng.standard_normal((D, D), dtype=np.float32) / math.sqrt(D)).astype(np.float32)
    b = (rng.standard_normal((D,), dtype=np.float32) * 0.01).astype(np.float32)
    o = kernel(x, k, W, b)
    print("ok", o.shape, o.dtype, float(np.abs(o).mean()))


# revision 21
# speedup vs baseline: 39.1266x; 32.1054x over previous
"""Trainium2 Bass kernel for nn_AttentionResidualBlock.

Computation (per token t, head h):
    q = x @ W_q + b_q
    scores[t,h,l] = <q[t,h,:], k[t,l,h,:]> / sqrt(hd)   (k = layer_history)
    w = softmax_l(scores)
    out[t,h,:] = sum_l w[t,h,l] * k[t,l,h,:]

Sharding: data-parallel over the 8192 (b,s) tokens -> 8 cores x 1024 tokens.
Per-core layout: token-major (tokens on partitions), 8 tiles of 128 tokens,
each tile processed as two independent head-halves (h 0:8 / 8:16) so the
serial softmax middle has fine grain and pipelines deeply across units.

v3 vs the 227us baseline:
  - all inputs fp16 on host (HBM traffic halves; accuracy improves ~10x)
  - all DMA on HWDGE queues; GPSIMD freed for compute
  - DVE (tensor_tensor 2x_1P) and GPSIMD (tensor_tensor, ~2ns/elem) split
    the fold tree and weighted-sum muls by layer (GF/GP2); TT ops never
    contend for the shared SBUF port
  - q PSUM->SBUF scale copy on ACT, per head-half right after its q_proj
  - head-half units + bufs>=3 pools give ~2 units of software-pipeline skew
"""

import math
from contextlib import ExitStack

import numpy as np

import concourse.tile as tile
from concourse import bacc, mybir
from concourse.bass_utils import run_bass_kernel_spmd
from concourse import masks

FP32 = mybir.dt.float32
FP16 = mybir.dt.float16

B, S, L, D, H = 4, 2048, 12, 1024, 16
HD = D // H
N_CORES = 8
T = B * S // N_CORES          # tokens per core = 1024
P = 128                       # partition tile
NT = T // P                   # 8 token tiles per core
SCALE = 1.0 / math.sqrt(HD)   # 0.125
HH = H // 2                   # heads per half = 8
DH = D // 2                   # feature cols per half = 512

# engine split tuning: layers handled by GPSIMD instead of DVE
GF = 0    # fold-tree layers on gpsimd (of L=12)
GP2 = 8   # weighted-sum mul layers on gpsimd (of L=12)


def build_body(ctx, tc, out, xt, kh, wq, bq, ones, repeat=1):
    nc = tc.nc

    const_pool = ctx.enter_context(tc.tile_pool(name="const", bufs=1))
    xtp = ctx.enter_context(tc.tile_pool(name="xt", bufs=2))
    xtr = xt.rearrange("(c p) t -> p c t", p=P)
    # tile 0's x^T slice first (it gates the first q_proj), then W half 0
    # (q_proj for heads 0:8 only needs W cols 0:512), then the rest
    xt0_sb = xtp.tile([P, 8, P], FP16, tag="xt")
    # W as lhsT chunks, one tile per half so q_proj h0 only waits on its own
    # half's DMA: w_sb[h][p, c, j] = W[c*128 + p, h*512 + j]
    wqr = wq.rearrange("(c p) j -> p c j", p=P)
    w_sb = [const_pool.tile([P, 8, DH], FP16, name=f"w{h}") for h in range(2)]
    kp = ctx.enter_context(tc.tile_pool(name="k", bufs=2))
    # prologue DMAs all on the scalar queue in dependence order: the DMA
    # engines drain one queue FIFO, so this ordering controls arrival
    k0_ch = [kp.tile([P, 4 if lc == 0 else 8, D], FP16, tag=f"k{lc}",
                     name=f"k0{lc}") for lc in range(2)]
    bq_sb = const_pool.tile([1, D], FP16)
    ones_sb = const_pool.tile([1, P], FP16)
    with tc.high_priority(offset=200):
        nc.scalar.dma_start(bq_sb[:], bq.unsqueeze(0))
        nc.scalar.dma_start(ones_sb[:], ones.unsqueeze(0))
        nc.scalar.dma_start(xt0_sb[:], xtr[:, :, 0:P])
        nc.scalar.dma_start(w_sb[0][:], wqr[:, :, 0:DH])
        nc.scalar.dma_start(k0_ch[0][:], kh[0:P, 0:4, :])
        nc.scalar.dma_start(w_sb[1][:], wqr[:, :, DH:D])
        nc.scalar.dma_start(k0_ch[1][:], kh[0:P, 4:12, :])
    ident = const_pool.tile([P, P], FP16)
    masks.make_identity(nc, ident[:])
    qp = ctx.enter_context(tc.tile_pool(name="q", bufs=2))
    prodp = ctx.enter_context(tc.tile_pool(name="prod", bufs=3))
    p2p = ctx.enter_context(tc.tile_pool(name="p2", bufs=1))
    op = ctx.enter_context(tc.tile_pool(name="o", bufs=1))
    sp = ctx.enter_context(tc.tile_pool(name="smx", bufs=4))
    ps_q = ctx.enter_context(tc.tile_pool(name="ps_q", bufs=2, space="PSUM"))
    ps_a = ctx.enter_context(tc.tile_pool(name="ps_a", bufs=2, space="PSUM"))

    # PE warm-up: ~4us of dummy matmuls at t~0 so the HAM clock-gate opens
    # before tile 0's q_proj (cold PE is the prologue critical path)
    warm_ps = ps_q.tile([P, DH], FP32, tag="q0")
    for i in range(64):
        nc.tensor.matmul(
            warm_ps[:, 0:P], lhsT=ident[:], rhs=ident[:],
            start=(i == 0), stop=(i == 63),
        )

    def flush_pending(pending):
        # one-tile-deferred output drain: by now the PE sum-over-l matmuls
        # for that tile are long done, so ACT never stalls on the PE counter
        a_prev, tok_prev = pending
        o_sb = op.tile([P, D], FP32, tag="o")
        nc.scalar.copy(o_sb[:], a_prev[:])
        nc.sync.dma_start(out[tok_prev], o_sb[:])

    pending = None
    for it in range(NT * repeat):
        tt = it % NT
        tok = slice(tt * P, (tt + 1) * P)

        # ---- loads ----
        if it == 0:
            xt_sb = xt0_sb
        else:
            xt_sb = xtp.tile([P, 8, P], FP16, tag="xt")
            nc.scalar.dma_start(xt_sb[:], xtr[:, :, tok])
        # k in 3 chunk tiles of 4 layers each: dependency tracking is
        # per-tile, so the first scores-mul only waits on its own chunk
        if it == 0:
            k_ch = k0_ch
        else:
            k_ch = [kp.tile([P, 4 if lc == 0 else 8, D], FP16, tag=f"k{lc}",
                            name=f"k{lc}") for lc in range(2)]
            nc.sync.dma_start(k_ch[0][:], kh[tok, 0:4, :])
            nc.sync.dma_start(k_ch[1][:], kh[tok, 4:12, :])

        acc = ps_a.tile([P, D], FP32, tag="acc")

        for hh in range(2):
            hcol = slice(hh * DH, (hh + 1) * DH)      # feature columns
            q_ps = ps_q.tile([P, DH], FP32, tag=f"q{hh}")
            # q_proj for this half's output columns (token-major PSUM)
            with tc.high_priority(offset=180):
                for c in range(8):
                    nc.tensor.matmul(
                        q_ps[:],
                        lhsT=xt_sb[:, c, :],
                        rhs=w_sb[hh][:, c, :],
                        start=(c == 0),
                        stop=False,
                    )
                nc.tensor.matmul(
                    q_ps[:],
                    lhsT=ones_sb[:],
                    rhs=bq_sb[:, hcol],
                    start=False,
                    stop=True,
                )
                # q -> SBUF fp16 with 1/sqrt(hd) folded in, on ACT
                q16 = qp.tile([P, DH], FP16, tag=f"q{hh}")
                nc.scalar.mul(q16[:], q_ps[:], SCALE)

            if pending is not None:
                flush_pending(pending)
                pending = None

            # ---- scores: prod = k * q (broadcast over l), fold over hd ----
            qhe = q16[:].rearrange("p (h e) -> p h e", h=HH).unsqueeze(1)
            qv4 = qhe.broadcast_to([P, 4, HH, HD])
            qv8 = qhe.broadcast_to([P, 8, HH, HD])
            prod = prodp.tile([P, L, HH, HD], FP16, tag=f"prod{hh}")
            scr = sp.tile([P, L, HH], FP32, tag=f"scr{hh}")

            def fold(eng, ls):
                # in-place fold tree over hd: 64->32->...->2, then fp32 tail.
                # dst aliases in1 exactly (same element positions) which is
                # safe for the streaming engines.
                off = 0
                for w0 in (32, 16, 8, 4, 2):
                    eng.tensor_add(
                        prod[:, ls, :, off + w0:off + 2 * w0],
                        prod[:, ls, :, off:off + w0],
                        prod[:, ls, :, off + w0:off + 2 * w0],
                    )
                    off += w0
                eng.tensor_add(
                    scr[:, ls].unsqueeze(3),
                    prod[:, ls, :, 62:63],
                    prod[:, ls, :, 63:64],
                )

            # last unit runs all-DVE so the kernel tail isn't gated on the
            # slower gpsimd ops
            last_unit = (it == NT * repeat - 1)
            gf = 0 if last_unit else GF
            gp2 = 0 if last_unit else GP2

            with tc.high_priority(offset=60):
                for lc in range(3):
                    k4c = k_ch[lc][:, :, hcol].rearrange(
                        "p l (h e) -> p l h e", h=HH)
                    nc.vector.tensor_mul(
                        prod[:, 4 * lc:4 * lc + 4], k4c, qv4)
                if gf > 0:
                    fold(nc.gpsimd, slice(0, gf))
                if gf < L:
                    fold(nc.vector, slice(gf, L))

            # ---- softmax over l (no max subtraction) ----
            es = sp.tile([P, L, HH], FP32, tag=f"es{hh}")
            nc.scalar.activation(es[:], scr[:], mybir.ActivationFunctionType.Exp)
            den = sp.tile([P, HH], FP32, tag=f"den{hh}")
            nc.vector.tensor_reduce(
                den[:],
                es[:].rearrange("p l h -> p h l"),
                axis=mybir.AxisListType.X,
                op=mybir.AluOpType.add,
            )
            rd = sp.tile([P, HH], FP32, tag=f"rd{hh}")
            nc.vector.reciprocal(rd[:], den[:])

            # normalized weights into slots 0,1 of the expanded tile (an
            # fp16 pair = one fp32 word), then fp32-word broadcast on ACT.
            # wb aliases prod (consumed by the fold); prod bufs=3 keeps the
            # resulting WAR two units away from the next mul
            wb = prod
            rdv = rd[:].unsqueeze(1).broadcast_to([P, L, HH]).unsqueeze(3)
            nc.vector.tensor_mul(
                wb[:, :, :, 0:2],
                es[:].unsqueeze(3).broadcast_to([P, L, HH, 2]),
                rdv.broadcast_to([P, L, HH, 2]),
            )
            wbf = wb[:].bitcast(FP32)  # [P, L, HH, 32] fp32 words
            prod2 = p2p.tile([P, L, DH], FP16, tag=f"p2{hh}")
            wbflat = wb[:].rearrange("p l h e -> p l (h e)")

            def wsum_mm(l, first, last):
                nc.tensor.matmul(
                    acc[:, hcol],
                    lhsT=ident[:],
                    rhs=prod2[:, l, :],
                    start=first,
                    stop=last,
                )

            def kk(l0, l1):
                # layers [l0, l1) must live in one k chunk tile (split 4+8)
                if l1 <= 4:
                    return k_ch[0][:, l0:l1, hcol]
                assert l0 >= 4
                return k_ch[1][:, l0 - 4:l1 - 4, hcol]

            # gpsimd takes layers [0, gp2) in 4-layer chunk ops; DVE the
            # rest in 2-layer groups pipelined with the expansion copies.
            for lh in range(6):
                ls = slice(lh * 2, (lh + 1) * 2)
                nc.scalar.copy(
                    wbf[:, ls, :, 1:32],
                    wbf[:, ls, :, 0:1].broadcast_to([P, 2, HH, 31]),
                )
                if ls.start < gp2:   # gp takes this 2-layer group
                    g1 = min(ls.stop, gp2)
                    nc.gpsimd.tensor_mul(
                        prod2[:, ls.start:g1, :], kk(ls.start, g1),
                        wbflat[:, ls.start:g1, :]
                    )
                d0 = max(ls.start, gp2)
                while d0 < ls.stop:
                    d1 = min(ls.stop, 4 if d0 < 4 else 12)
                    nc.vector.tensor_mul(
                        prod2[:, d0:d1, :], kk(d0, d1), wbflat[:, d0:d1, :]
                    )
                    d0 = d1
                for l in range(max(ls.start, gp2), ls.stop):
                    wsum_mm(l, first=(l == gp2), last=(l == L - 1 and gp2 == 0))
            for l in range(gp2):
                wsum_mm(l, first=(gp2 == L), last=(l == gp2 - 1))

        pending = (acc, tok)

    flush_pending(pending)


_NC_CACHE = {}


def build_nc(repeat=1):
    if repeat in _NC_CACHE:
        return _NC_CACHE[repeat]
    nc = bacc.Bacc("TRN2", target_bir_lowering=False, debug=False,
                   num_devices=N_CORES)
    xt = nc.dram_tensor("xt", [D, T], FP16, kind="ExternalInput").ap()
    kh = nc.dram_tensor("kh", [T, L, D], FP16, kind="ExternalInput").ap()
    wq = nc.dram_tensor("wq", [D, D], FP16, kind="ExternalInput").ap()
    bq = nc.dram_tensor("bq", [D], FP16, kind="ExternalInput").ap()
    ones = nc.dram_tensor("ones", [P], FP16, kind="ExternalInput").ap()
    out = nc.dram_tensor("out", [T, D], FP32, kind="ExternalOutput").ap()
    with tile.TileContext(nc) as tc, ExitStack() as ctx:
        build_body(ctx, tc, out, xt, kh, wq, bq, ones, repeat=repeat)
    nc.compile()
    _NC_CACHE[repeat] = nc
    return nc


def make_in_maps(x_current, layer_history, W_q, b_q):
    x16 = np.asarray(x_current, dtype=np.float16).reshape(B * S, D)
    k16 = np.asarray(layer_history, dtype=np.float16).reshape(B * S, L, D)
    W16 = np.asarray(W_q, dtype=np.float16)
    b16 = np.asarray(b_q, dtype=np.float16)
    in_maps = []
    for c in range(N_CORES):
        sl = slice(c * T, (c + 1) * T)
        in_maps.append({
            "xt": np.ascontiguousarray(x16[sl].T),
            "kh": np.ascontiguousarray(k16[sl]),
            "wq": W16,
            "bq": b16,
            "ones": np.ones((P,), np.float16),
        })
    return in_maps


def kernel(x_current, layer_history, W_q, b_q):
    nc = build_nc()
    in_maps = make_in_maps(x_current, layer_history, W_q, b_q)
    res = run_bass_kernel_spmd(nc, in_maps, core_ids=list(range(N_CORES)))
    out = np.concatenate([res.results[c]["out"] for c in range(N_CORES)], axis=0)
    return out.reshape(B, S, D).astype(np.float32)


if __name__ == "__main__":
    rng = np.random.default_rng(0)
    x = rng.standard_normal((B, S, D), dtype=np.float32)
    k = rng.standard_normal((B, S, L, D), dtype=np.float32)
    W = (rng.standard_normal((D, D), dtype=np.float32) / math.sqrt(D)).astype(np.float32)
    b = (rng.standard_normal((D,), dtype=np.float32) * 0.01).astype(np.float32)
    o = kernel(x, k, W, b)
    print("ok", o.shape, o.dtype, float(np.abs(o).mean()))
